# revision 16
# baseline (speedup 1.0000x reference)
"""Trainium2 Bass kernel for nn_Attention_30305289240928.

Single-layer causal attention with RMSNorm prologue:
    xn = x * rsqrt(mean(x^2) + eps)           (RMSNorm, no weight)
    qkv = xn @ wqkv.T  -> per-head q, k, v    (16 heads, head_dim 128)
    out = softmax(causal(q k^T / sqrt(128))) v, concat heads, @ wo.T

Sharding: head-parallel tensor parallel over 8 NeuronCores.
Core c owns heads 2c, 2c+1 (wqkv rows c*768:(c+1)*768) and the matching
wo input-columns c*256:(c+1)*256. Each core computes a full-shape partial
of the output projection (rank-256 contribution); the host sums the 8
partials (the TP all-reduce, done host-side at gather time).

Device-side design:
  - QKV projection and the RMSNorm sum-of-squares run as fp8e4 DoubleRow
    matmuls (0.5 cycles/row, 256-deep contraction per instruction).
    Accuracy is preserved with residual compensation: the host ships
    x8=e4m3(8 x), xr8=e4m3(8 x - x8) and w8=e4m3(512 w), wr8 likewise;
    qkv = x8 w8 + xr8 w8 + x8 wr8 (the xr8*wr8 term is ~1e-4, dropped).
    Measured end-to-end relative error ~4.0e-3 (gate 2e-2).
  - The output projection is also 3-term fp8 DoubleRow: attnT is written
    at 16x (folded into sT) and quantized to e4m3 + residual by one ACT
    copy and one DVE scalar_tensor_tensor per (qb, head); wo ships
    pre-quantized (512x) with its residual. The 8192x product folds into
    the output eviction multiplier.
  - The 4096x scale factor and the RMSNorm scale s[t] fold into existing
    eviction/exp constants; the attention core stays float32r (fp8 es is
    blocked by exp overflow without a per-column running max).
  - ssq = ones-DR-matmul over fp8 squares of x8 (split ACT/DVE, emitted
    one tb ahead so the matmuls never wait on the elementwise stream).
    The systematic ssq deficit (missing cross/xr^2 terms, e4m3 rounding
    skew of chi^2-distributed squares) is corrected by an exact
    host-computed mean folded into the sqrt bias.
  - Scores are computed transposed, S.T[kt, qt], so the softmax-exp output
    feeds the PV matmul directly; causal masking = N-sliced matmuls plus a
    triangular multiplicative mask on diagonal blocks.
  - sum-of-exp via ones-matmul accumulated in PSUM alongside PV.
  - DMA discipline: the input ramp uses few large transfers (each DMA
    costs ~650ns of SP sequencer + HWDGE time; the ramp is
    bandwidth-bound otherwise); tb0 runs term-major so only the
    w8/x8-dependent matmuls wait on the leading stream.
  - Phase 2 runs query blocks in DESCENDING order with each output
    projection interleaved one attention head behind, the final two
    output blocks' DMA groups alternated, and the output written in bf16
    (host sums partials in f32), so the final DMA drain is short.
"""

import numpy as np
import ml_dtypes

import concourse.bacc as bacc
import concourse.mybir as mybir
import concourse.tile as tile
from concourse import bass_utils

# Problem shapes (hardcoded per contract)
S = 2048          # sequence length
H = 2048          # hidden
NH = 16           # heads
D = 128           # head dim
EPS = 1e-5
N_CORES = 8
HPC = NH // N_CORES        # heads per core = 2
FPC = 3 * D * HPC          # wqkv features per core = 768
CPC = D * HPC              # attn dims (wo input cols) per core = 256

TB = 256                   # token block width (phase 1)
NTB = S // TB              # 8
NM = TB // 128             # 128-wide sub-blocks per token block
NHO = H // 128             # 16 hidden 128-chunks
NHP = NHO // 2             # 8 hidden 256-pairs (DoubleRow k-tiles)
QB = 512                   # query block width (phase 2)
NQB = S // QB              # 4
NKB = S // 128             # 16 key 128-blocks
SQRT_D_INV = 1.0 / float(np.sqrt(D))

AX = 8.0                   # host scale on x before e4m3
AW = 512.0                 # host scale on wqkv before e4m3
AXW = AX * AW              # 4096: folded out at PSUM eviction
AO = 16.0                  # attnT scale (via sT) before e4m3
AWO = 512.0                # host scale on wo before e4m3
# squares: sq = (x8 * SQ_SCALE)^2 = 8 x^2  (max ~162, fits e4m3)
SQ_SCALE = float(1.0 / (2.0 * np.sqrt(2.0)))
# sqrt_t = sqrt(ps_ssq * SQRT_SCALE + SQRT_BIAS) = 4096 sqrt(ssq/H + eps)
SQRT_SCALE = AXW * AXW / (8.0 * H)
SQRT_BIAS = AXW * AXW * EPS

f32 = mybir.dt.float32
f32r = mybir.dt.float32r
bf16 = mybir.dt.bfloat16
f8 = mybir.dt.float8e4
DR = mybir.MatmulPerfMode.DoubleRow
E4 = ml_dtypes.float8_e4m3
BF16 = ml_dtypes.bfloat16

_CACHED_NC = None


def _build():
    nc = bacc.Bacc("TRN2", target_bir_lowering=False, debug=False,
                   num_devices=N_CORES)
    x8_d = nc.dram_tensor("x8", [H, S], f8, kind="ExternalInput").ap()
    xr8_d = nc.dram_tensor("xr8", [H, S], f8, kind="ExternalInput").ap()
    w8_d = nc.dram_tensor("w8", [H, FPC], f8, kind="ExternalInput").ap()
    wr8_d = nc.dram_tensor("wr8", [H, FPC], f8, kind="ExternalInput").ap()
    wo8_d = nc.dram_tensor("wo8", [CPC, S], f8, kind="ExternalInput").ap()
    wor8_d = nc.dram_tensor("wor8", [CPC, S], f8, kind="ExternalInput").ap()
    # cst = [ones(128,128) | zeros(128,128) | tri_upper(128,128) | eye(128,128)]
    cst_d = nc.dram_tensor("cst", [128, 512], f32, kind="ExternalInput").ap()
    # oneh[p, c*128+i] = (p == c): bf16 one-hot selectors for the rse
    # partition->free broadcast matmuls
    oneh_d = nc.dram_tensor("oneh", [4, 512], bf16, kind="ExternalInput").ap()
    # [ones(256) | 0.25(256)]: 1.0 for sq8 DR, 0.25 rescales the cross term
    ones8_d = nc.dram_tensor("ones8", [128, 512], f8, kind="ExternalInput").ap()
    # per-problem sqrt bias: 4096^2*eps + mean_t(sum_h xr^2) correction
    bias_d = nc.dram_tensor("biasb", [128, 1], f32, kind="ExternalInput").ap()
    outT_d = nc.dram_tensor("outT", [H, S], bf16, kind="ExternalOutput").ap()

    with tile.TileContext(nc) as tc:
        with tc.tile_pool(name="const", bufs=1) as const_pool, \
             tc.tile_pool(name="qk", bufs=1) as qk_pool, \
             tc.tile_pool(name="vsb", bufs=1) as v_pool, \
             tc.tile_pool(name="attn", bufs=1) as attn_pool, \
             tc.tile_pool(name="svec", bufs=1) as s_pool:

            ones_r = const_pool.tile([128, 128], f32r, tag="ones")
            ones8 = const_pool.tile([128, 2, 2, 128], f8, tag="ones8")
            zt = const_pool.tile([128, 256], f32, tag="zt")   # [zeros | tri]
            tri = zt[:, 128:256]
            eye = const_pool.tile([128, 128], f32, tag="eye")
            eps_b = const_pool.tile([128, 1], f32, tag="eps")
            oneh = const_pool.tile([4, 512], bf16, tag="oneh")

            # phase-1 outputs (live into phases 2/3)
            qkT = qk_pool.tile([128, 2 * HPC, S], f32r)   # [q0,k0,q1,k1] x S
            v_sb = v_pool.tile([128, NKB, CPC], f32r)     # V natural, t-chunked
            attnT = attn_pool.tile([128, HPC, S], f32r)   # O.T rows (this core)
            s_bc = s_pool.tile([128, NTB, TB], f32)       # s[t]/4096, bcast
            sTd = s_pool.tile([128, NKB], f32)            # s[t]/sqrt(D), t parts
            sT = s_pool.tile([128, NKB], f32)             # s[t]/4096, t on parts

            # ---------------- Phase 1: RMSNorm stats + QKV projection ------
            with tc.tile_pool(name="wt", bufs=1) as wt_pool, \
                 tc.tile_pool(name="xt", bufs=2) as xt_pool, \
                 tc.tile_pool(name="sq", bufs=2) as sq_pool, \
                 tc.tile_pool(name="ph1", bufs=2) as ph1_pool, \
                 tc.tile_pool(name="ps_qk", bufs=4, space="PSUM") as psum_qk, \
                 tc.tile_pool(name="ps_v", bufs=2, space="PSUM") as psum_v, \
                 tc.tile_pool(name="ps_ssq", bufs=1, space="PSUM") as psum_ssq, \
                 tc.tile_pool(name="ps_t", bufs=1, space="PSUM") as psum_t:

                # weights: [128, ho, 768] with feature order
                # [q0 k0 q1 k1 v0 v1]; pairs of ho chunks feed DoubleRow
                w8 = wt_pool.tile([128, NHO, FPC], f8, tag="w8")
                wr8 = wt_pool.tile([128, NHO, FPC], f8, tag="wr8")

                def load_w(wtile, dram, ho0, nho):
                    nc.sync.dma_start(
                        wtile[:, ho0:ho0 + nho],
                        dram[ho0 * 128:(ho0 + nho) * 128, :]
                        .rearrange("(ho p) f -> p ho f", p=128))

                def load_x(xb, halves=False):
                    """one 512-token block of x8 + xr8 (>=512B runs);
                    halves=True splits by ho so the consuming tb's first
                    DR pairs unblock at half-transfer."""
                    t8 = xt_pool.tile([128, NHO, 2 * TB], f8, tag="x8")
                    tr = xt_pool.tile([128, NHO, 2 * TB], f8, tag="xr8")
                    for dram, t in ((x8_d, t8), (xr8_d, tr)):
                        nq = 2 if halves else 1
                        hq = NHO // nq
                        for q in range(nq):
                            nc.sync.dma_start(
                                t[:, q * hq:(q + 1) * hq],
                                dram[q * hq * 128:(q + 1) * hq * 128,
                                     xb * 512:(xb + 1) * 512]
                                .rearrange("(ho p) t -> p ho t", p=128))
                    return t8, tr

                def load_x_quarter(t, dram, xb, q):
                    hq = NHO // 4
                    nc.sync.dma_start(
                        t[:, q * hq:(q + 1) * hq],
                        dram[q * hq * 128:(q + 1) * hq * 128,
                             xb * 512:(xb + 1) * 512]
                        .rearrange("(ho p) t -> p ho t", p=128))

                # ramp: each DMA pays ~650ns of SP sequencer + HWDGE time,
                # so the ramp uses few, large transfers; w8 quarter-loads
                # interleave with x8 quarters (the first K block's chain),
                # xr8 (term 2) and wr8 (term 3) stream behind in halves.
                t8 = xt_pool.tile([128, NHO, 2 * TB], f8, tag="x8")
                tr = xt_pool.tile([128, NHO, 2 * TB], f8, tag="xr8")
                x_cur = (t8, tr)
                for q in range(4):
                    load_w(w8, w8_d, 4 * q, 4)
                    load_x_quarter(t8, x8_d, 0, q)
                    if q == 0:
                        nc.sync.dma_start(ones8[:], ones8_d.rearrange(
                            "p (c two f) -> p c two f", c=2, two=2))
                        nc.sync.dma_start(eps_b[:], bias_d)
                    if q == 1:
                        nc.sync.dma_start(ones_r[:], cst_d[:, 0:128].bitcast(f32r))
                # small consts next (eye gates tb0's s-transposes), then
                # xr8 fully before wr8: tb0 consumes all term-2 (xr8) pairs
                # before its first term-3 (wr8) matmul
                nc.sync.dma_start(zt[:], cst_d[:, 128:384])
                nc.sync.dma_start(eye[:], cst_d[:, 384:512])
                nc.sync.dma_start(oneh[:], oneh_d)
                for hf in range(2):
                    nc.sync.dma_start(
                        tr[:, 8 * hf:8 * hf + 8],
                        xr8_d[hf * 1024:(hf + 1) * 1024, 0:512]
                        .rearrange("(ho p) t -> p ho t", p=128))
                for hf in range(2):
                    load_w(wr8, wr8_d, 8 * hf, 8)

                def emit_squares(tb_, xt_pair):
                    # sq8 = 8 x8^2 (ACT, 4-chunk batched) in e4m3, DR-matmul'd
                    # against ones. The systematic deficit (missing cross/xr^2
                    # terms and the e4m3 rounding skew of the squares) is a
                    # host-side constant folded into the sqrt bias; the
                    # per-token residual is ~1e-3 on s. Emitted one tb AHEAD
                    # so the ssq matmuls never wait on this stream.
                    x8_, _ = xt_pair
                    h_ = (tb_ % 2) * TB
                    sq8_ = sq_pool.tile([128, NHO, TB], f8, tag="sq")
                    for g in range(2):
                        nc.scalar.activation(
                            sq8_[:, 4 * g:4 * g + 4],
                            x8_[:, 4 * g:4 * g + 4, h_:h_ + TB],
                            mybir.ActivationFunctionType.Square,
                            scale=SQ_SCALE)
                    for g in range(2, 4):
                        nc.vector.scalar_tensor_tensor(
                            sq8_[:, 4 * g:4 * g + 4],
                            x8_[:, 4 * g:4 * g + 4, h_:h_ + TB], 0.125,
                            x8_[:, 4 * g:4 * g + 4, h_:h_ + TB],
                            mybir.AluOpType.mult, mybir.AluOpType.mult)
                    return sq8_

                sq_cur = emit_squares(0, x_cur)
                for tb in range(NTB):
                    x8t, xr8t = x_cur
                    sq8 = sq_cur
                    half = (tb % 2) * TB
                    if tb % 2 == 1 and tb + 1 < NTB:
                        x_next = load_x((tb + 1) // 2)

                    # term operand pairs: qk blocks use (w, x), V uses (x, w)
                    qk_terms = ((w8, x8t), (w8, xr8t), (wr8, x8t))
                    v_terms = ((x8t, w8), (xr8t, w8), (x8t, wr8))

                    def qk_term(ps, fb, t, start, stop):
                        wtile, xtile = qk_terms[t]
                        fs = slice(fb * 128, (fb + 1) * 128)
                        for hp in range(NHP):
                            nc.tensor.matmul(
                                ps[:], wtile[:, 2 * hp:2 * hp + 2, fs],
                                xtile[:, 2 * hp:2 * hp + 2, half:half + TB],
                                start=(start and hp == 0),
                                stop=(stop and hp == NHP - 1), perf_mode=DR)

                    def qk_evict(ps, slot):
                        dst = qkT[:, slot, tb * TB:(tb + 1) * TB]
                        if slot in (0, 2):   # Q: x s/4096 during eviction
                            nc.vector.tensor_tensor(dst, ps[:], s_bc[:, tb],
                                                    mybir.AluOpType.mult)
                        else:                # K: undo the 4096 host scale
                            nc.scalar.mul(dst, ps[:], 1.0 / AXW)

                    def v_term(ps, m, t, start, stop):
                        xtile, wtile = v_terms[t]
                        ts = slice(half + m * 128, half + (m + 1) * 128)
                        for hp in range(NHP):
                            nc.tensor.matmul(
                                ps[:], xtile[:, 2 * hp:2 * hp + 2, ts],
                                wtile[:, 2 * hp:2 * hp + 2, 512:768],
                                start=(start and hp == 0),
                                stop=(stop and hp == NHP - 1), perf_mode=DR)

                    def v_evict(ps, m):
                        chunk = tb * NM + m
                        nc.vector.tensor_scalar_mul(
                            v_sb[:, chunk], ps[:], sT[:, chunk:chunk + 1])

                    def ssq_term(ps, start, stop):
                        for hp in range(NHP):
                            nc.tensor.matmul(ps[:], ones8[:, 0],
                                             sq8[:, 2 * hp:2 * hp + 2],
                                             start=(start and hp == 0),
                                             stop=(stop and hp == NHP - 1),
                                             perf_mode=DR)

                    def s_chain(ps_ssq):
                        # s/4096 = 1/(4096 sqrt(ssq/H + eps))
                        sqrt_t = ph1_pool.tile([128, TB], f32, tag="sqrt")
                        nc.scalar.activation(sqrt_t[:], ps_ssq[:],
                                             mybir.ActivationFunctionType.Sqrt,
                                             bias=eps_b[:], scale=SQRT_SCALE)
                        nc.vector.reciprocal_approx_fast(s_bc[:, tb], sqrt_t[:])

                    def s_transpose(m):
                        pt = psum_t.tile([128, 128], f32)
                        nc.tensor.transpose(
                            pt[:], s_bc[:, tb, m * 128:(m + 1) * 128], eye[:])
                        col = tb * NM + m
                        nc.scalar.mul(sTd[:, col:col + 1], pt[:, 0:1],
                                      AXW * SQRT_D_INV)
                        # 16x folds into V so attnT lands in e4m3's sweet
                        # spot for the fp8 output projection
                        nc.scalar.mul(sT[:, col:col + 1], pt[:, 0:1], AO)

                    if tb == 0:
                        # term-major: all w8-only matmuls run first so PE is
                        # never blocked on the trailing xr8/wr8 DMA streams
                        pk0 = psum_qk.tile([128, TB], f32, tag="qk")
                        pk1 = psum_qk.tile([128, TB], f32, tag="qk")
                        pq0 = psum_qk.tile([128, TB], f32, tag="qk")
                        pq1 = psum_qk.tile([128, TB], f32, tag="qk")
                        pv0 = psum_v.tile([128, CPC], f32, tag="v")
                        pv1 = psum_v.tile([128, CPC], f32, tag="v")
                        pss = psum_ssq.tile([128, TB], f32, tag="ssq")
                        blocks = [(pk0, 1), (pk1, 3), (pq0, 0), (pq1, 2)]
                        for ps, fb in blocks:
                            qk_term(ps, fb, 0, True, False)
                        v_term(pv0, 0, 0, True, False)
                        v_term(pv1, 1, 0, True, False)
                        for ps, fb in blocks:
                            qk_term(ps, fb, 1, False, False)
                        ssq_term(pss, True, True)
                        s_chain(pss)
                        v_term(pv0, 0, 1, False, False)
                        v_term(pv1, 1, 1, False, False)
                        qk_term(pk0, 1, 2, False, True)
                        qk_evict(pk0, 1)
                        qk_term(pk1, 3, 2, False, True)
                        qk_evict(pk1, 3)
                        qk_term(pq0, 0, 2, False, True)
                        qk_evict(pq0, 0)
                        qk_term(pq1, 2, 2, False, True)
                        qk_evict(pq1, 2)
                        s_transpose(0)
                        s_transpose(1)
                        v_term(pv0, 0, 2, False, True)
                        v_evict(pv0, 0)
                        v_term(pv1, 1, 2, False, True)
                        v_evict(pv1, 1)
                    else:
                        # steady state: K blocks evict immediately; Q blocks
                        # run their matmuls before the ssq matmuls (which wait
                        # on the DVE cross ops), with evictions deferred until
                        # s is ready, so PE never sits on the s chain.
                        # ssq first: sq8 was produced during the previous tb,
                        # so the s chain hides under the K blocks
                        pss = psum_ssq.tile([128, TB], f32, tag="ssq")
                        ssq_term(pss, True, True)
                        s_chain(pss)
                        for slot, fb in ((1, 1), (3, 3)):
                            ps = psum_qk.tile([128, TB], f32, tag="qk")
                            for t in range(3):
                                qk_term(ps, fb, t, t == 0, t == 2)
                            qk_evict(ps, slot)
                        for slot, fb in ((0, 0), (2, 2)):
                            ps = psum_qk.tile([128, TB], f32, tag="qk")
                            for t in range(3):
                                qk_term(ps, fb, t, t == 0, t == 2)
                            qk_evict(ps, slot)
                        s_transpose(0)
                        s_transpose(1)
                        for m in range(NM):
                            ps = psum_v.tile([128, CPC], f32, tag="v")
                            for t in range(3):
                                v_term(ps, m, t, t == 0, t == 2)
                            v_evict(ps, m)

                    if tb + 1 < NTB:
                        nxt = x_next if tb % 2 == 1 else x_cur
                        sq_cur = emit_squares(tb + 1, nxt)
                        if tb % 2 == 1:
                            x_cur = x_next

            # -------- Phase 2+3: attention (qb-desc) + output projection ---
            with tc.tile_pool(name="wo", bufs=1) as wo_pool, \
                 tc.tile_pool(name="exps", bufs=8) as exp_pool, \
                 tc.tile_pool(name="rse", bufs=2) as rse_pool, \
                 tc.tile_pool(name="ostage", bufs=8) as out_pool, \
                 tc.tile_pool(name="ps_s", bufs=3, space="PSUM") as psum_s, \
                 tc.tile_pool(name="ps_o", bufs=2, space="PSUM") as psum_o, \
                 tc.tile_pool(name="ps_se", bufs=1, space="PSUM") as psum_rse, \
                 tc.tile_pool(name="ps_out", bufs=2, space="PSUM") as psum_out:
                # wo.T streams in while early attention runs (fp8 + residual)
                wo8 = wo_pool.tile([128, HPC, S], f8, tag="wo8")
                wor8 = wo_pool.tile([128, HPC, S], f8, tag="wor8")
                nc.sync.dma_start(
                    wo8[:], wo8_d.rearrange("(ch p) o -> p ch o", p=128))
                nc.sync.dma_start(
                    wor8[:], wor8_d.rearrange("(ch p) o -> p ch o", p=128))
                # fp8 attnT (16x-scaled via sT) + residual for the 3-term
                # DoubleRow output projection
                attnT8 = wo_pool.tile([128, HPC, S], f8, tag="a8")
                attnr8 = wo_pool.tile([128, HPC, S], f8, tag="ar8")

                def attn_head(qb, h):
                    kb_hi = (qb + 1) * (QB // 128) - 1
                    q_slot, k_slot = 2 * h, 2 * h + 1
                    po = psum_o.tile([128, QB], f32)
                    # one bank, three consecutive lives: cols 0:4 accumulate
                    # the per-q-chunk sum-of-exp (es-as-stationary matmuls,
                    # ap_size=1 so PE engine time ~0), cols 128:256 hold the
                    # [4,128] transpose, then the bf16 one-hot broadcasts
                    # overwrite the full bank with rse replicated across
                    # partitions. Tile's slice tracking serializes the lives.
                    rt = psum_rse.tile([128, QB], f32)
                    for kb in range(kb_hi + 1):
                        j = kb - qb * (QB // 128)  # >=0 in diagonal zone
                        # j==3 pads the active range to N=256 (fp32r is
                        # 4x slower below 256); the extra below-diagonal
                        # strip is zeroed by the widened [zeros|tri] mask
                        lo = 256 if j == 3 else max(0, j) * 128
                        ps = psum_s.tile([128, QB], f32)
                        nc.tensor.matmul(
                            ps[:, lo:],
                            qkT[:, k_slot, kb * 128:(kb + 1) * 128],
                            qkT[:, q_slot, qb * QB + lo:(qb + 1) * QB],
                            start=True, stop=True)
                        es = exp_pool.tile([128, QB], f32r)
                        nc.scalar.activation(
                            es[:, lo:], ps[:, lo:],
                            mybir.ActivationFunctionType.Exp,
                            scale=sTd[:, kb:kb + 1])
                        if j == 3:
                            nc.vector.tensor_tensor(
                                es[:, 256:512],
                                es[:, 256:512].bitcast(f32),
                                zt[:], mybir.AluOpType.mult)
                        elif j >= 0:
                            nc.vector.tensor_tensor(
                                es[:, j * 128:(j + 1) * 128],
                                es[:, j * 128:(j + 1) * 128].bitcast(f32),
                                tri[:], mybir.AluOpType.mult)
                        nc.tensor.matmul(
                            po[:, lo:], v_sb[:, kb, h * D:(h + 1) * D],
                            es[:, lo:], start=(kb == 0), stop=(kb == kb_hi))
                        # sum-of-exp per 128-query chunk: es chunk is the
                        # stationary, a ones column the moving, so the whole
                        # partition-dim reduction costs ~1 output row
                        for c in range(lo // 128, 4):
                            c_last = qb * (QB // 128) + c if c < 2 else kb_hi
                            nc.tensor.matmul(
                                rt[:, c:c + 1], es[:, c * 128:(c + 1) * 128],
                                ones_r[:, 0:1],
                                start=(kb == 0), stop=(kb == c_last))
                    rse4 = rse_pool.tile([128, 4], f32, tag="rse4")
                    nc.vector.reciprocal_approx_fast(rse4[:], rt[:, 0:4])
                    # partition->free flip of the 4 rse columns, then bf16
                    # one-hot matmuls replicate each chunk across partitions
                    nc.tensor.transpose(rt[0:4, 128:256], rse4[:], eye[:])
                    rrow = rse_pool.tile([4, 128], bf16, tag="rrow")
                    nc.scalar.copy(rrow[:], rt[0:4, 128:256])
                    for c in range(4):
                        nc.tensor.matmul(
                            rt[:, c * 128:(c + 1) * 128],
                            oneh[:, c * 128:(c + 1) * 128], rrow[:],
                            start=True, stop=True)
                    qs = slice(qb * QB, (qb + 1) * QB)
                    nc.vector.tensor_tensor(
                        attnT[:, h, qs], po[:], rt[:],
                        mybir.AluOpType.mult)
                    # e4m3 quantize on Pool (idle engine); residual on DVE
                    nc.gpsimd.tensor_scalar(
                        attnT8[:, h, qs], attnT[:, h, qs].bitcast(f32),
                        0.0, None, mybir.AluOpType.bypass)
                    nc.vector.scalar_tensor_tensor(
                        attnr8[:, h, qs], attnT8[:, h, qs], -1.0,
                        attnT[:, h, qs].bitcast(f32),
                        mybir.AluOpType.mult, mybir.AluOpType.add)

                def outproj(sb, gs=0, ge=8, borrow=False, fine=False):
                    # evacs land in a 2-block staging tile; one DMA per group
                    # (fine=True: one DMA per ob so the final drain is short).
                    # PSUM values are written RAW (the AO*AWO fold is divided
                    # out host-side after the partial sum); even obs evict on
                    # the otherwise-idle Pool engine, odd obs on DVE, keeping
                    # ACT free for the exp stream.
                    for g in range(gs, ge):
                        st = out_pool.tile([128, 2, 512], bf16, tag="ost")
                        for oi in range(2):
                            ob = g * 2 + oi
                            if borrow and ob % 2 == 0:
                                ps = psum_s.tile([128, QB], f32)
                            else:
                                ps = psum_out.tile([128, 512], f32)
                            terms = ((attnT8, wo8), (attnr8, wo8),
                                     (attnT8, wor8))
                            for t, (a_t, w_t) in enumerate(terms):
                                nc.tensor.matmul(
                                    ps[:], w_t[:, :, ob * 128:(ob + 1) * 128],
                                    a_t[:, :, sb * 512:(sb + 1) * 512],
                                    start=(t == 0), stop=(t == 2),
                                    perf_mode=DR)
                            # split eviction: two half-width copies on
                            # different engines run in parallel, halving the
                            # PSUM-bank release latency (the WAR that gates
                            # the next ob's matmul group); rotate engine
                            # pairs so DVE/ACT/Pool each take 2/3 of obs
                            e0, e1 = ((nc.vector, nc.gpsimd) if ob % 2 == 0
                                      else (nc.gpsimd, nc.vector))
                            for eng, hs in ((e0, slice(0, 256)),
                                            (e1, slice(256, 512))):
                                if eng is nc.scalar:
                                    nc.scalar.copy(st[:, oi, hs], ps[:, hs])
                                else:
                                    eng.tensor_scalar(
                                        st[:, oi, hs], ps[:, hs], 0.0, None,
                                        mybir.AluOpType.bypass)
                            if fine:
                                nc.sync.dma_start(
                                    outT_d[ob * 128:(ob + 1) * 128,
                                           sb * 512:(sb + 1) * 512]
                                    .rearrange("(ob p) t -> p ob t", p=128),
                                    st[:, oi:oi + 1])
                        if not fine:
                            nc.sync.dma_start(
                                outT_d[g * 256:(g + 1) * 256,
                                       sb * 512:(sb + 1) * 512]
                                .rearrange("(ob p) t -> p ob t", p=128), st[:])

                # descending qb: the largest attention blocks run first, each
                # outproj(sb) is emitted one head after attn(sb, 1) so the
                # pse->recip->attnT chain stays off PE's critical path, and
                # the tail is the smallest block + a short bf16 DMA drain.
                attn_head(3, 0)
                attn_head(3, 1)
                outproj(3, 0, 4)
                attn_head(2, 0)
                outproj(3, 4, 8)
                attn_head(2, 1)
                outproj(2, 0, 4)
                attn_head(1, 0)
                outproj(2, 4, 8)
                attn_head(1, 1)
                outproj(1, 0, 4)
                attn_head(0, 0)
                outproj(1, 4, 6)
                attn_head(0, 1)
                # alternate the two remaining output blocks so their DMAs
                # spread across the full PE stream instead of piling up
                for g in range(2):
                    outproj(0, g, g + 1, borrow=True)
                    outproj(1, 6 + g, 7 + g, borrow=True)
                outproj(0, 2, 8, borrow=True)
    nc.compile()
    return nc


def get_nc():
    global _CACHED_NC
    if _CACHED_NC is None:
        _CACHED_NC = _build()
    return _CACHED_NC


def make_in_maps(x, wqkv, wo):
    x = np.asarray(x, dtype=np.float32)
    wqkv = np.asarray(wqkv, dtype=np.float32)
    wo = np.asarray(wo, dtype=np.float32)

    xs = np.ascontiguousarray(x.T) * AX           # [H, S]
    x8 = xs.astype(E4)
    xr8 = (xs - x8.astype(np.float32)).astype(E4)

    cst = np.concatenate(
        [np.ones((128, 128), np.float32),
         np.zeros((128, 128), np.float32),
         np.triu(np.ones((128, 128), np.float32)),
         np.eye(128, dtype=np.float32)], axis=1)
    ones8 = np.concatenate(
        [np.ones((128, 256), np.float32),
         np.full((128, 256), 0.25, np.float32)], axis=1).astype(E4)
    oneh = np.zeros((4, 512), dtype=np.float32)
    for c in range(4):
        oneh[c, c * 128:(c + 1) * 128] = 1.0
    oneh = oneh.astype(BF16)
    # The device ssq = sum(sq8) carries a systematic deficit: the missing
    # 2 x xr cross term, the missing xr^2 term, and the e4m3 rounding bias
    # of the squares (chi^2 density falls steeply across each 12.5%-wide
    # fp8 bin, so round-to-nearest skews low). Fold the exact mean deficit
    # into the sqrt bias; the per-token residual is ~1e-3 relative on s.
    x8f = x8.astype(np.float32)
    sq8 = ((x8f * SQ_SCALE) ** 2).astype(E4).astype(np.float32)
    ps_model = sq8.sum(axis=0)
    ps_true = 8.0 * (x.T ** 2).sum(axis=0)
    deficit = (ps_true - ps_model).mean()
    biasb = np.full((128, 1), SQRT_BIAS + deficit * SQRT_SCALE,
                    dtype=np.float32)

    in_maps = []
    for c in range(N_CORES):
        wc = wqkv[c * FPC:(c + 1) * FPC]          # [768, H] rows h*384+j
        # reorder rows to [q0 k0 q1 k1 v0 v1] (128 each)
        order = np.concatenate([
            np.arange(0, 128), np.arange(128, 256),        # q0 k0
            np.arange(384, 512), np.arange(512, 640),      # q1 k1
            np.arange(256, 384), np.arange(640, 768)])     # v0 v1
        ws = np.ascontiguousarray(wc[order].T) * AW        # [H, 768]
        w8 = ws.astype(E4)
        wr8 = (ws - w8.astype(np.float32)).astype(E4)
        wos = np.ascontiguousarray(wo[:, c * CPC:(c + 1) * CPC].T) * AWO
        wo8 = wos.astype(E4)
        wor8 = (wos - wo8.astype(np.float32)).astype(E4)
        in_maps.append({"x8": x8, "xr8": xr8, "w8": w8, "wr8": wr8,
                        "wo8": wo8, "wor8": wor8, "cst": cst,
                        "ones8": ones8, "biasb": biasb, "oneh": oneh})
    return in_maps


def kernel(x, wqkv, wo):
    nc = get_nc()
    in_maps = make_in_maps(x, wqkv, wo)
    res = None
    for attempt in range(4):
        try:
            res = bass_utils.run_bass_kernel_spmd(
                nc, in_maps, core_ids=list(range(N_CORES)))
            break
        except Exception:
            # transient NRT device wedges have been observed; they recover
            # after a short quiescent period, so back off before retrying
            if attempt == 3:
                raise
            import time
            time.sleep(20 * (attempt + 1))
    outT = np.zeros((H, S), dtype=np.float32)
    for c in range(N_CORES):
        outT += np.asarray(res.results[c]["outT"]).astype(np.float32)
    outT *= 1.0 / (AO * AWO)
    return np.ascontiguousarray(outT.T)



# revision 17
# speedup vs baseline: 1.0009x; 1.0009x over previous
"""Trainium2 Bass kernel for nn_Attention_30305289240928.

Single-layer causal attention with RMSNorm prologue:
    xn = x * rsqrt(mean(x^2) + eps)           (RMSNorm, no weight)
    qkv = xn @ wqkv.T  -> per-head q, k, v    (16 heads, head_dim 128)
    out = softmax(causal(q k^T / sqrt(128))) v, concat heads, @ wo.T

Sharding: head-parallel tensor parallel over 8 NeuronCores.
Core c owns heads 2c, 2c+1 (wqkv rows c*768:(c+1)*768) and the matching
wo input-columns c*256:(c+1)*256. Each core computes a full-shape partial
of the output projection (rank-256 contribution); the host sums the 8
partials (the TP all-reduce, done host-side at gather time).

Device-side design:
  - QKV projection and the RMSNorm sum-of-squares run as fp8e4 DoubleRow
    matmuls (0.5 cycles/row, 256-deep contraction per instruction).
    Accuracy is preserved with residual compensation: the host ships
    x8=e4m3(8 x), xr8=e4m3(8 x - x8) and w8=e4m3(512 w), wr8 likewise;
    qkv = x8 w8 + xr8 w8 + x8 wr8 (the xr8*wr8 term is ~1e-4, dropped).
    Measured end-to-end relative error ~4.0e-3 (gate 2e-2).
  - The output projection is also 3-term fp8 DoubleRow: attnT is written
    at 16x (folded into sT) and quantized to e4m3 + residual by one ACT
    copy and one DVE scalar_tensor_tensor per (qb, head); wo ships
    pre-quantized (512x) with its residual. The 8192x product folds into
    the output eviction multiplier.
  - The 4096x scale factor and the RMSNorm scale s[t] fold into existing
    eviction/exp constants; the attention core stays float32r (fp8 es is
    blocked by exp overflow without a per-column running max).
  - ssq = ones-DR-matmul over fp8 squares of x8 (split ACT/DVE, emitted
    one tb ahead so the matmuls never wait on the elementwise stream).
    The systematic ssq deficit (missing cross/xr^2 terms, e4m3 rounding
    skew of chi^2-distributed squares) is corrected by an exact
    host-computed mean folded into the sqrt bias.
  - Scores are computed transposed, S.T[kt, qt], so the softmax-exp output
    feeds the PV matmul directly; causal masking = N-sliced matmuls plus a
    triangular multiplicative mask on diagonal blocks.
  - sum-of-exp via ones-matmul accumulated in PSUM alongside PV.
  - DMA discipline: the input ramp uses few large transfers (each DMA
    costs ~650ns of SP sequencer + HWDGE time; the ramp is
    bandwidth-bound otherwise); tb0 runs term-major so only the
    w8/x8-dependent matmuls wait on the leading stream.
  - Phase 2 runs query blocks in DESCENDING order with each output
    projection interleaved one attention head behind, the final two
    output blocks' DMA groups alternated, and the output written in bf16
    (host sums partials in f32), so the final DMA drain is short.
"""

import numpy as np
import ml_dtypes

import concourse.bacc as bacc
import concourse.mybir as mybir
import concourse.tile as tile
from concourse import bass_utils

# Problem shapes (hardcoded per contract)
S = 2048          # sequence length
H = 2048          # hidden
NH = 16           # heads
D = 128           # head dim
EPS = 1e-5
N_CORES = 8
HPC = NH // N_CORES        # heads per core = 2
FPC = 3 * D * HPC          # wqkv features per core = 768
CPC = D * HPC              # attn dims (wo input cols) per core = 256

TB = 256                   # token block width (phase 1)
NTB = S // TB              # 8
NM = TB // 128             # 128-wide sub-blocks per token block
NHO = H // 128             # 16 hidden 128-chunks
NHP = NHO // 2             # 8 hidden 256-pairs (DoubleRow k-tiles)
QB = 512                   # query block width (phase 2)
NQB = S // QB              # 4
NKB = S // 128             # 16 key 128-blocks
SQRT_D_INV = 1.0 / float(np.sqrt(D))

AX = 8.0                   # host scale on x before e4m3
AW = 512.0                 # host scale on wqkv before e4m3
AXW = AX * AW              # 4096: folded out at PSUM eviction
AO = 16.0                  # attnT scale (via sT) before e4m3
AWO = 512.0                # host scale on wo before e4m3
# squares: sq = (x8 * SQ_SCALE)^2 = 8 x^2  (max ~162, fits e4m3)
SQ_SCALE = float(1.0 / (2.0 * np.sqrt(2.0)))
# sqrt_t = sqrt(ps_ssq * SQRT_SCALE + SQRT_BIAS) = 4096 sqrt(ssq/H + eps)
SQRT_SCALE = AXW * AXW / (8.0 * H)
SQRT_BIAS = AXW * AXW * EPS

f32 = mybir.dt.float32
f32r = mybir.dt.float32r
bf16 = mybir.dt.bfloat16
f8 = mybir.dt.float8e4
DR = mybir.MatmulPerfMode.DoubleRow
E4 = ml_dtypes.float8_e4m3
BF16 = ml_dtypes.bfloat16

_CACHED_NC = None


def _build():
    nc = bacc.Bacc("TRN2", target_bir_lowering=False, debug=False,
                   num_devices=N_CORES)
    x8_d = nc.dram_tensor("x8", [H, S], f8, kind="ExternalInput").ap()
    xr8_d = nc.dram_tensor("xr8", [H, S], f8, kind="ExternalInput").ap()
    w8_d = nc.dram_tensor("w8", [H, FPC], f8, kind="ExternalInput").ap()
    wr8_d = nc.dram_tensor("wr8", [H, FPC], f8, kind="ExternalInput").ap()
    wo8_d = nc.dram_tensor("wo8", [CPC, S], f8, kind="ExternalInput").ap()
    wor8_d = nc.dram_tensor("wor8", [CPC, S], f8, kind="ExternalInput").ap()
    # cst = [ones(128,128) | zeros(128,128) | tri_upper(128,128) | eye(128,128)]
    cst_d = nc.dram_tensor("cst", [128, 512], f32, kind="ExternalInput").ap()
    # oneh[p, c*128+i] = (p == c): bf16 one-hot selectors for the rse
    # partition->free broadcast matmuls
    oneh_d = nc.dram_tensor("oneh", [4, 512], bf16, kind="ExternalInput").ap()
    # [ones(256) | 0.25(256)]: 1.0 for sq8 DR, 0.25 rescales the cross term
    ones8_d = nc.dram_tensor("ones8", [128, 512], f8, kind="ExternalInput").ap()
    # per-problem sqrt bias: 4096^2*eps + mean_t(sum_h xr^2) correction
    bias_d = nc.dram_tensor("biasb", [128, 1], f32, kind="ExternalInput").ap()
    outT_d = nc.dram_tensor("outT", [H, S], bf16, kind="ExternalOutput").ap()

    with tile.TileContext(nc) as tc:
        with tc.tile_pool(name="const", bufs=1) as const_pool, \
             tc.tile_pool(name="qk", bufs=1) as qk_pool, \
             tc.tile_pool(name="vsb", bufs=1) as v_pool, \
             tc.tile_pool(name="attn", bufs=1) as attn_pool, \
             tc.tile_pool(name="svec", bufs=1) as s_pool:

            ones_r = const_pool.tile([128, 128], f32r, tag="ones")
            ones8 = const_pool.tile([128, 2, 2, 128], f8, tag="ones8")
            zt = const_pool.tile([128, 256], f32, tag="zt")   # [zeros | tri]
            tri = zt[:, 128:256]
            eye = const_pool.tile([128, 128], f32, tag="eye")
            eps_b = const_pool.tile([128, 1], f32, tag="eps")
            oneh = const_pool.tile([4, 512], bf16, tag="oneh")

            # phase-1 outputs (live into phases 2/3)
            qkT = qk_pool.tile([128, 2 * HPC, S], f32r)   # [q0,k0,q1,k1] x S
            v_sb = v_pool.tile([128, NKB, CPC], f32r)     # V natural, t-chunked
            attnT = attn_pool.tile([128, HPC, S], f32r)   # O.T rows (this core)
            s_bc = s_pool.tile([128, NTB, TB], f32)       # s[t]/4096, bcast
            sTd = s_pool.tile([128, NKB], f32)            # s[t]/sqrt(D), t parts
            sT = s_pool.tile([128, NKB], f32)             # s[t]/4096, t on parts

            # ---------------- Phase 1: RMSNorm stats + QKV projection ------
            with tc.tile_pool(name="wt", bufs=1) as wt_pool, \
                 tc.tile_pool(name="xt", bufs=2) as xt_pool, \
                 tc.tile_pool(name="sq", bufs=2) as sq_pool, \
                 tc.tile_pool(name="ph1", bufs=2) as ph1_pool, \
                 tc.tile_pool(name="ps_qk", bufs=4, space="PSUM") as psum_qk, \
                 tc.tile_pool(name="ps_v", bufs=2, space="PSUM") as psum_v, \
                 tc.tile_pool(name="ps_ssq", bufs=1, space="PSUM") as psum_ssq, \
                 tc.tile_pool(name="ps_t", bufs=1, space="PSUM") as psum_t:

                # weights: [128, ho, 768] with feature order
                # [q0 k0 q1 k1 v0 v1]; pairs of ho chunks feed DoubleRow
                w8 = wt_pool.tile([128, NHO, FPC], f8, tag="w8")
                wr8 = wt_pool.tile([128, NHO, FPC], f8, tag="wr8")

                def load_w(wtile, dram, ho0, nho):
                    nc.sync.dma_start(
                        wtile[:, ho0:ho0 + nho],
                        dram[ho0 * 128:(ho0 + nho) * 128, :]
                        .rearrange("(ho p) f -> p ho f", p=128))

                def load_x(xb, halves=False):
                    """one 512-token block of x8 + xr8 (>=512B runs);
                    halves=True splits by ho so the consuming tb's first
                    DR pairs unblock at half-transfer."""
                    t8 = xt_pool.tile([128, NHO, 2 * TB], f8, tag="x8")
                    tr = xt_pool.tile([128, NHO, 2 * TB], f8, tag="xr8")
                    for dram, t in ((x8_d, t8), (xr8_d, tr)):
                        nq = 2 if halves else 1
                        hq = NHO // nq
                        for q in range(nq):
                            nc.sync.dma_start(
                                t[:, q * hq:(q + 1) * hq],
                                dram[q * hq * 128:(q + 1) * hq * 128,
                                     xb * 512:(xb + 1) * 512]
                                .rearrange("(ho p) t -> p ho t", p=128))
                    return t8, tr

                def load_x_quarter(t, dram, xb, q):
                    hq = NHO // 4
                    nc.sync.dma_start(
                        t[:, q * hq:(q + 1) * hq],
                        dram[q * hq * 128:(q + 1) * hq * 128,
                             xb * 512:(xb + 1) * 512]
                        .rearrange("(ho p) t -> p ho t", p=128))

                # ramp: each DMA pays ~650ns of SP sequencer + HWDGE time,
                # so the ramp uses few, large transfers; w8 quarter-loads
                # interleave with x8 quarters (the first K block's chain),
                # xr8 (term 2) and wr8 (term 3) stream behind in halves.
                t8 = xt_pool.tile([128, NHO, 2 * TB], f8, tag="x8")
                tr = xt_pool.tile([128, NHO, 2 * TB], f8, tag="xr8")
                x_cur = (t8, tr)
                for q in range(4):
                    load_w(w8, w8_d, 4 * q, 4)
                    load_x_quarter(t8, x8_d, 0, q)
                    if q == 0:
                        nc.sync.dma_start(ones8[:], ones8_d.rearrange(
                            "p (c two f) -> p c two f", c=2, two=2))
                        nc.sync.dma_start(eps_b[:], bias_d)
                    if q == 1:
                        nc.sync.dma_start(ones_r[:], cst_d[:, 0:128].bitcast(f32r))
                # small consts next (eye gates tb0's s-transposes), then
                # xr8 fully before wr8: tb0 consumes all term-2 (xr8) pairs
                # before its first term-3 (wr8) matmul
                nc.sync.dma_start(zt[:], cst_d[:, 128:384])
                nc.sync.dma_start(eye[:], cst_d[:, 384:512])
                nc.sync.dma_start(oneh[:], oneh_d)
                for hf in range(2):
                    nc.sync.dma_start(
                        tr[:, 8 * hf:8 * hf + 8],
                        xr8_d[hf * 1024:(hf + 1) * 1024, 0:512]
                        .rearrange("(ho p) t -> p ho t", p=128))
                for hf in range(2):
                    load_w(wr8, wr8_d, 8 * hf, 8)

                def emit_squares(tb_, xt_pair):
                    # sq8 = 8 x8^2 (ACT, 4-chunk batched) in e4m3, DR-matmul'd
                    # against ones. The systematic deficit (missing cross/xr^2
                    # terms and the e4m3 rounding skew of the squares) is a
                    # host-side constant folded into the sqrt bias; the
                    # per-token residual is ~1e-3 on s. Emitted one tb AHEAD
                    # so the ssq matmuls never wait on this stream.
                    x8_, _ = xt_pair
                    h_ = (tb_ % 2) * TB
                    sq8_ = sq_pool.tile([128, NHO, TB], f8, tag="sq")
                    for g in range(2):
                        nc.scalar.activation(
                            sq8_[:, 4 * g:4 * g + 4],
                            x8_[:, 4 * g:4 * g + 4, h_:h_ + TB],
                            mybir.ActivationFunctionType.Square,
                            scale=SQ_SCALE)
                    for g in range(2, 4):
                        nc.vector.scalar_tensor_tensor(
                            sq8_[:, 4 * g:4 * g + 4],
                            x8_[:, 4 * g:4 * g + 4, h_:h_ + TB], 0.125,
                            x8_[:, 4 * g:4 * g + 4, h_:h_ + TB],
                            mybir.AluOpType.mult, mybir.AluOpType.mult)
                    return sq8_

                sq_cur = emit_squares(0, x_cur)
                for tb in range(NTB):
                    x8t, xr8t = x_cur
                    sq8 = sq_cur
                    half = (tb % 2) * TB
                    if tb % 2 == 1 and tb + 1 < NTB:
                        x_next = load_x((tb + 1) // 2)

                    # term operand pairs: qk blocks use (w, x), V uses (x, w)
                    qk_terms = ((w8, x8t), (w8, xr8t), (wr8, x8t))
                    v_terms = ((x8t, w8), (xr8t, w8), (x8t, wr8))

                    def qk_term(ps, fb, t, start, stop):
                        wtile, xtile = qk_terms[t]
                        fs = slice(fb * 128, (fb + 1) * 128)
                        for hp in range(NHP):
                            nc.tensor.matmul(
                                ps[:], wtile[:, 2 * hp:2 * hp + 2, fs],
                                xtile[:, 2 * hp:2 * hp + 2, half:half + TB],
                                start=(start and hp == 0),
                                stop=(stop and hp == NHP - 1), perf_mode=DR)

                    def qk_evict(ps, slot):
                        dst = qkT[:, slot, tb * TB:(tb + 1) * TB]
                        if slot in (0, 2):   # Q: x s/4096 during eviction
                            nc.vector.tensor_tensor(dst, ps[:], s_bc[:, tb],
                                                    mybir.AluOpType.mult)
                        else:                # K: undo the 4096 host scale
                            nc.scalar.mul(dst, ps[:], 1.0 / AXW)

                    def v_term(ps, m, t, start, stop):
                        xtile, wtile = v_terms[t]
                        ts = slice(half + m * 128, half + (m + 1) * 128)
                        for hp in range(NHP):
                            nc.tensor.matmul(
                                ps[:], xtile[:, 2 * hp:2 * hp + 2, ts],
                                wtile[:, 2 * hp:2 * hp + 2, 512:768],
                                start=(start and hp == 0),
                                stop=(stop and hp == NHP - 1), perf_mode=DR)

                    def v_evict(ps, m):
                        chunk = tb * NM + m
                        nc.vector.tensor_scalar_mul(
                            v_sb[:, chunk], ps[:], sT[:, chunk:chunk + 1])

                    def ssq_term(ps, start, stop):
                        for hp in range(NHP):
                            nc.tensor.matmul(ps[:], ones8[:, 0],
                                             sq8[:, 2 * hp:2 * hp + 2],
                                             start=(start and hp == 0),
                                             stop=(stop and hp == NHP - 1),
                                             perf_mode=DR)

                    def s_chain(ps_ssq):
                        # s/4096 = 1/(4096 sqrt(ssq/H + eps))
                        sqrt_t = ph1_pool.tile([128, TB], f32, tag="sqrt")
                        nc.scalar.activation(sqrt_t[:], ps_ssq[:],
                                             mybir.ActivationFunctionType.Sqrt,
                                             bias=eps_b[:], scale=SQRT_SCALE)
                        nc.vector.reciprocal_approx_fast(s_bc[:, tb], sqrt_t[:])

                    def s_transpose(m):
                        pt = psum_t.tile([128, 128], f32)
                        nc.tensor.transpose(
                            pt[:], s_bc[:, tb, m * 128:(m + 1) * 128], eye[:])
                        col = tb * NM + m
                        nc.scalar.mul(sTd[:, col:col + 1], pt[:, 0:1],
                                      AXW * SQRT_D_INV)
                        # 16x folds into V so attnT lands in e4m3's sweet
                        # spot for the fp8 output projection
                        nc.scalar.mul(sT[:, col:col + 1], pt[:, 0:1], AO)

                    if tb == 0:
                        # term-major: all w8-only matmuls run first so PE is
                        # never blocked on the trailing xr8/wr8 DMA streams
                        pk0 = psum_qk.tile([128, TB], f32, tag="qk")
                        pk1 = psum_qk.tile([128, TB], f32, tag="qk")
                        pq0 = psum_qk.tile([128, TB], f32, tag="qk")
                        pq1 = psum_qk.tile([128, TB], f32, tag="qk")
                        pv0 = psum_v.tile([128, CPC], f32, tag="v")
                        pv1 = psum_v.tile([128, CPC], f32, tag="v")
                        pss = psum_ssq.tile([128, TB], f32, tag="ssq")
                        blocks = [(pk0, 1), (pk1, 3), (pq0, 0), (pq1, 2)]
                        for ps, fb in blocks:
                            qk_term(ps, fb, 0, True, False)
                        v_term(pv0, 0, 0, True, False)
                        v_term(pv1, 1, 0, True, False)
                        for ps, fb in blocks:
                            qk_term(ps, fb, 1, False, False)
                        ssq_term(pss, True, True)
                        s_chain(pss)
                        v_term(pv0, 0, 1, False, False)
                        v_term(pv1, 1, 1, False, False)
                        qk_term(pk0, 1, 2, False, True)
                        qk_evict(pk0, 1)
                        qk_term(pk1, 3, 2, False, True)
                        qk_evict(pk1, 3)
                        qk_term(pq0, 0, 2, False, True)
                        qk_evict(pq0, 0)
                        qk_term(pq1, 2, 2, False, True)
                        qk_evict(pq1, 2)
                        s_transpose(0)
                        s_transpose(1)
                        v_term(pv0, 0, 2, False, True)
                        v_evict(pv0, 0)
                        v_term(pv1, 1, 2, False, True)
                        v_evict(pv1, 1)
                    else:
                        # steady state: K blocks evict immediately; Q blocks
                        # run their matmuls before the ssq matmuls (which wait
                        # on the DVE cross ops), with evictions deferred until
                        # s is ready, so PE never sits on the s chain.
                        # ssq first: sq8 was produced during the previous tb,
                        # so the s chain hides under the K blocks
                        pss = psum_ssq.tile([128, TB], f32, tag="ssq")
                        ssq_term(pss, True, True)
                        s_chain(pss)
                        for slot, fb in ((1, 1), (3, 3)):
                            ps = psum_qk.tile([128, TB], f32, tag="qk")
                            for t in range(3):
                                qk_term(ps, fb, t, t == 0, t == 2)
                            qk_evict(ps, slot)
                        for slot, fb in ((0, 0), (2, 2)):
                            ps = psum_qk.tile([128, TB], f32, tag="qk")
                            for t in range(3):
                                qk_term(ps, fb, t, t == 0, t == 2)
                            qk_evict(ps, slot)
                        s_transpose(0)
                        s_transpose(1)
                        for m in range(NM):
                            ps = psum_v.tile([128, CPC], f32, tag="v")
                            for t in range(3):
                                v_term(ps, m, t, t == 0, t == 2)
                            v_evict(ps, m)

                    if tb + 1 < NTB:
                        nxt = x_next if tb % 2 == 1 else x_cur
                        sq_cur = emit_squares(tb + 1, nxt)
                        if tb % 2 == 1:
                            x_cur = x_next

            # -------- Phase 2+3: attention (qb-desc) + output projection ---
            with tc.tile_pool(name="wo", bufs=1) as wo_pool, \
                 tc.tile_pool(name="exps", bufs=8) as exp_pool, \
                 tc.tile_pool(name="rse", bufs=2) as rse_pool, \
                 tc.tile_pool(name="ostage", bufs=8) as out_pool, \
                 tc.tile_pool(name="ps_s", bufs=3, space="PSUM") as psum_s, \
                 tc.tile_pool(name="ps_o", bufs=2, space="PSUM") as psum_o, \
                 tc.tile_pool(name="ps_se", bufs=1, space="PSUM") as psum_rse, \
                 tc.tile_pool(name="ps_out", bufs=2, space="PSUM") as psum_out:
                # wo.T streams in while early attention runs (fp8 + residual)
                wo8 = wo_pool.tile([128, HPC, S], f8, tag="wo8")
                wor8 = wo_pool.tile([128, HPC, S], f8, tag="wor8")
                nc.sync.dma_start(
                    wo8[:], wo8_d.rearrange("(ch p) o -> p ch o", p=128))
                nc.sync.dma_start(
                    wor8[:], wor8_d.rearrange("(ch p) o -> p ch o", p=128))
                # fp8 attnT (16x-scaled via sT) + residual for the 3-term
                # DoubleRow output projection
                attnT8 = wo_pool.tile([128, HPC, S], f8, tag="a8")
                attnr8 = wo_pool.tile([128, HPC, S], f8, tag="ar8")

                # Deferred-task queue: outproj obs and rse-finisher steps are
                # emitted BETWEEN attention kb iterations ("pumped"), so each
                # PE instruction reaches the head of the in-order PE queue
                # with its cross-engine inputs (recip, rrow, attnr8,
                # evictions) already computed, and PSUM-bank WAR release
                # latency is hidden by the kb-paced spacing.
                tasks = []

                def pump(n=1):
                    for _ in range(n):
                        if tasks:
                            tasks.pop(0)()

                def attn_head(qb, h):
                    kb_hi = (qb + 1) * (QB // 128) - 1
                    q_slot, k_slot = 2 * h, 2 * h + 1
                    po = psum_o.tile([128, QB], f32)
                    # one bank, three consecutive lives: cols 0:4 accumulate
                    # the per-q-chunk sum-of-exp (es-as-stationary matmuls,
                    # ap_size=1 so PE engine time ~0), cols 128:256 hold the
                    # [4,128] transpose, then the bf16 one-hot broadcasts
                    # overwrite the full bank with rse replicated across
                    # partitions. Tile's slice tracking serializes the lives.
                    rt = psum_rse.tile([128, QB], f32)
                    for kb in range(kb_hi + 1):
                        j = kb - qb * (QB // 128)  # >=0 in diagonal zone
                        # j==3 pads the active range to N=256 (fp32r is
                        # 4x slower below 256); the extra below-diagonal
                        # strip is zeroed by the widened [zeros|tri] mask
                        lo = 256 if j == 3 else max(0, j) * 128
                        ps = psum_s.tile([128, QB], f32)
                        nc.tensor.matmul(
                            ps[:, lo:],
                            qkT[:, k_slot, kb * 128:(kb + 1) * 128],
                            qkT[:, q_slot, qb * QB + lo:(qb + 1) * QB],
                            start=True, stop=True)
                        es = exp_pool.tile([128, QB], f32r)
                        nc.scalar.activation(
                            es[:, lo:], ps[:, lo:],
                            mybir.ActivationFunctionType.Exp,
                            scale=sTd[:, kb:kb + 1])
                        if j == 3:
                            nc.vector.tensor_tensor(
                                es[:, 256:512],
                                es[:, 256:512].bitcast(f32),
                                zt[:], mybir.AluOpType.mult)
                        elif j >= 0:
                            nc.vector.tensor_tensor(
                                es[:, j * 128:(j + 1) * 128],
                                es[:, j * 128:(j + 1) * 128].bitcast(f32),
                                tri[:], mybir.AluOpType.mult)
                        nc.tensor.matmul(
                            po[:, lo:], v_sb[:, kb, h * D:(h + 1) * D],
                            es[:, lo:], start=(kb == 0), stop=(kb == kb_hi))
                        # sum-of-exp per 128-query chunk: es chunk is the
                        # stationary, a ones column the moving, so the whole
                        # partition-dim reduction costs ~1 output row
                        for c in range(lo // 128, 4):
                            c_last = qb * (QB // 128) + c if c < 2 else kb_hi
                            nc.tensor.matmul(
                                rt[:, c:c + 1], es[:, c * 128:(c + 1) * 128],
                                ones_r[:, 0:1],
                                start=(kb == 0), stop=(kb == c_last))
                        if kb >= 1:
                            pump(1 if qb >= 2 else 2)
                    rse4 = rse_pool.tile([128, 4], f32, tag="rse4")
                    nc.vector.reciprocal_approx_fast(rse4[:], rt[:, 0:4])
                    rrow = rse_pool.tile([4, 128], bf16, tag="rrow")

                    def fin1():
                        # partition->free flip of the 4 rse columns
                        nc.tensor.transpose(rt[0:4, 128:256], rse4[:], eye[:])
                        nc.vector.tensor_scalar(
                            rrow[:], rt[0:4, 128:256], 0.0, None,
                            mybir.AluOpType.bypass)

                    def fin2():
                        # bf16 one-hot matmuls replicate rse across partitions
                        for c in range(4):
                            nc.tensor.matmul(
                                rt[:, c * 128:(c + 1) * 128],
                                oneh[:, c * 128:(c + 1) * 128], rrow[:],
                                start=True, stop=True)
                        qs = slice(qb * QB, (qb + 1) * QB)
                        nc.vector.tensor_tensor(
                            attnT[:, h, qs], po[:], rt[:],
                            mybir.AluOpType.mult)
                        # e4m3 quantize on Pool (idle engine); resid on DVE
                        nc.gpsimd.tensor_scalar(
                            attnT8[:, h, qs], attnT[:, h, qs].bitcast(f32),
                            0.0, None, mybir.AluOpType.bypass)
                        nc.vector.scalar_tensor_tensor(
                            attnr8[:, h, qs], attnT8[:, h, qs], -1.0,
                            attnT[:, h, qs].bitcast(f32),
                            mybir.AluOpType.mult, mybir.AluOpType.add)

                    tasks.append(fin1)
                    tasks.append(fin2)

                def ob_task(sb, ob, borrow=False):
                    # one output block: 3-term fp8 DR matmul group, whole-ob
                    # eviction on Pool/DVE (PSUM raw; AO*AWO divided out
                    # host-side), per-ob DMA so the drain stays fine-grained
                    def run():
                        if borrow:
                            ps = psum_s.tile([128, QB], f32)
                        else:
                            ps = psum_out.tile([128, 512], f32)
                        terms = ((attnT8, wo8), (attnr8, wo8),
                                 (attnT8, wor8))
                        for t, (a_t, w_t) in enumerate(terms):
                            nc.tensor.matmul(
                                ps[:], w_t[:, :, ob * 128:(ob + 1) * 128],
                                a_t[:, :, sb * 512:(sb + 1) * 512],
                                start=(t == 0), stop=(t == 2),
                                perf_mode=DR)
                        st = out_pool.tile([128, 512], bf16, tag="ost")
                        eng = nc.gpsimd if ob % 2 == 0 else nc.vector
                        eng.tensor_scalar(st[:], ps[:], 0.0, None,
                                          mybir.AluOpType.bypass)
                        nc.sync.dma_start(
                            outT_d[ob * 128:(ob + 1) * 128,
                                   sb * 512:(sb + 1) * 512]
                            .rearrange("(ob p) t -> p ob t", p=128), st[:])
                    return run

                def queue_outproj(sb, borrow=False):
                    for ob in range(16):
                        tasks.append(ob_task(sb, ob, borrow))

                # descending qb: the largest attention blocks run first; the
                # task pump interleaves each sb's output blocks and each
                # head's rse finishers into the FOLLOWING heads' kb loops,
                # with the tail (sb=0) drained at the end on freed PSUM
                attn_head(3, 0)
                attn_head(3, 1)
                queue_outproj(3)
                attn_head(2, 0)
                attn_head(2, 1)
                queue_outproj(2)
                attn_head(1, 0)
                attn_head(1, 1)
                queue_outproj(1)
                attn_head(0, 0)
                attn_head(0, 1)
                queue_outproj(0, borrow=True)
                while tasks:
                    pump()
    nc.compile()
    return nc


def get_nc():
    global _CACHED_NC
    if _CACHED_NC is None:
        _CACHED_NC = _build()
    return _CACHED_NC


def make_in_maps(x, wqkv, wo):
    x = np.asarray(x, dtype=np.float32)
    wqkv = np.asarray(wqkv, dtype=np.float32)
    wo = np.asarray(wo, dtype=np.float32)

    xs = np.ascontiguousarray(x.T) * AX           # [H, S]
    x8 = xs.astype(E4)
    xr8 = (xs - x8.astype(np.float32)).astype(E4)

    cst = np.concatenate(
        [np.ones((128, 128), np.float32),
         np.zeros((128, 128), np.float32),
         np.triu(np.ones((128, 128), np.float32)),
         np.eye(128, dtype=np.float32)], axis=1)
    ones8 = np.concatenate(
        [np.ones((128, 256), np.float32),
         np.full((128, 256), 0.25, np.float32)], axis=1).astype(E4)
    oneh = np.zeros((4, 512), dtype=np.float32)
    for c in range(4):
        oneh[c, c * 128:(c + 1) * 128] = 1.0
    oneh = oneh.astype(BF16)
    # The device ssq = sum(sq8) carries a systematic deficit: the missing
    # 2 x xr cross term, the missing xr^2 term, and the e4m3 rounding bias
    # of the squares (chi^2 density falls steeply across each 12.5%-wide
    # fp8 bin, so round-to-nearest skews low). Fold the exact mean deficit
    # into the sqrt bias; the per-token residual is ~1e-3 relative on s.
    x8f = x8.astype(np.float32)
    sq8 = ((x8f * SQ_SCALE) ** 2).astype(E4).astype(np.float32)
    ps_model = sq8.sum(axis=0)
    ps_true = 8.0 * (x.T ** 2).sum(axis=0)
    deficit = (ps_true - ps_model).mean()
    biasb = np.full((128, 1), SQRT_BIAS + deficit * SQRT_SCALE,
                    dtype=np.float32)

    in_maps = []
    for c in range(N_CORES):
        wc = wqkv[c * FPC:(c + 1) * FPC]          # [768, H] rows h*384+j
        # reorder rows to [q0 k0 q1 k1 v0 v1] (128 each)
        order = np.concatenate([
            np.arange(0, 128), np.arange(128, 256),        # q0 k0
            np.arange(384, 512), np.arange(512, 640),      # q1 k1
            np.arange(256, 384), np.arange(640, 768)])     # v0 v1
        ws = np.ascontiguousarray(wc[order].T) * AW        # [H, 768]
        w8 = ws.astype(E4)
        wr8 = (ws - w8.astype(np.float32)).astype(E4)
        wos = np.ascontiguousarray(wo[:, c * CPC:(c + 1) * CPC].T) * AWO
        wo8 = wos.astype(E4)
        wor8 = (wos - wo8.astype(np.float32)).astype(E4)
        in_maps.append({"x8": x8, "xr8": xr8, "w8": w8, "wr8": wr8,
                        "wo8": wo8, "wor8": wor8, "cst": cst,
                        "ones8": ones8, "biasb": biasb, "oneh": oneh})
    return in_maps


def kernel(x, wqkv, wo):
    nc = get_nc()
    in_maps = make_in_maps(x, wqkv, wo)
    res = None
    for attempt in range(4):
        try:
            res = bass_utils.run_bass_kernel_spmd(
                nc, in_maps, core_ids=list(range(N_CORES)))
            break
        except Exception:
            # transient NRT device wedges have been observed; they recover
            # after a short quiescent period, so back off before retrying
            if attempt == 3:
                raise
            import time
            time.sleep(20 * (attempt + 1))
    outT = np.zeros((H, S), dtype=np.float32)
    for c in range(N_CORES):
        outT += np.asarray(res.results[c]["outT"]).astype(np.float32)
    outT *= 1.0 / (AO * AWO)
    return np.ascontiguousarray(outT.T)

